# revision 23
# baseline (speedup 1.0000x reference)
"""VQ codebook nearest-embedding kernel for 8x Trainium2 NeuronCores.

Computes, for x (32, 256, 32, 32) and emb (256, 2048):
    xf = moveaxis(x, 1, -1).reshape(-1, 256)           # (N=32768, 256)
    argmin_n = argmin_k ||xf_n - emb[:, k]||^2
    out[n] = emb[:, argmin_n]   -> reshaped back to (32, 256, 32, 32)

Sharding: data-parallel over the batch dim (4 batches per core, N_c = 4096
latent positions per core); the codebook is replicated.

Key layout fact: x[b] is (D=256, H*W=1024) contiguous, which is exactly the
lhsT (K=D on partitions, M=latents on free) layout the PE wants -- no
transposes anywhere.  argmin_k d2 == argmax_k (x.e_k - 0.5*||e_k||^2); the
bias is folded into the PSUM accumulation group as a K=1 ones-outer-product
matmul, so per 128-latent tile the full (128, 2048) score block lands in
PSUM off pure matmuls.  ScalarE copies scores to SBUF, VectorE max +
max_index extract the argmax, and an indirect DMA gathers the winning
codebook rows.
"""

import os
import sys

import numpy as np

if os.path.isdir("/opt/trn_rl_repo") and "/opt/trn_rl_repo" not in sys.path:
    sys.path.append("/opt/trn_rl_repo")

B, D, H, W = 32, 256, 32, 32
K = 2048
NCORES = 8
BPC = B // NCORES          # batches per core
HW = H * W                 # latents per batch
NPC = BPC * HW             # latents per core
LT = 128                   # latent tile (PSUM partition dim)
NLT = NPC // LT            # latent tiles per core
CB = 512                   # code block (PSUM bank free dim)
NCB = K // CB              # code blocks

# "fp32": native fp32 matmuls (4 cyc/row).
# "fp22x3": hi/lo split operands, 3 float32r passes (~fp32 precision, 3 cyc/row).
# "fp22": single float32r pass (1 cyc/row); coarse scores carry ~1e-3-scale
#   rounding error, so rows whose top-2 score margin falls below TAU are
#   re-argmin'd exactly on the host from the original fp32 data.
MODE = os.environ.get("VQ_KERNEL_MODE", "fp22x3")
TAU = float(os.environ.get("VQ_TAU", "0.05"))
# keep-warm matmuls per code block: PE HAM re-throttles to 1.2 GHz when the
# array's duty cycle drops in DVE-bound phases; dummy N=512 matmuls into a
# scratch PSUM bank keep it at 2.4 GHz.
NDUMMY = int(os.environ.get("VQ_DUMMY", "2"))

_cache = {}


def _fp22_round(a: np.ndarray) -> np.ndarray:
    """Round fp32 mantissa to 11 bits -- the PE float32r operand-read
    precision (HW-probed: RTN-even keeping mantissa bits >= 2^-11; products
    of two such values are exact in the fp32 accumulation)."""
    b = a.astype(np.float32).view(np.uint32).astype(np.uint64)
    r = ((b + 0x800) & 0xFFFFF000).astype(np.uint32)
    return r.view(np.float32)


def _split_hi_lo(a: np.ndarray):
    hi = _fp22_round(a)
    lo = (a.astype(np.float32) - hi).astype(np.float32)
    return hi, lo


def _split_excess_waits(nc, mybir, maxw: int = 1):
    """Walrus's per-instruction encodings have very few sem-wait slots.
    Move all but `maxw` of each instruction's waits onto same-engine NOPs
    inserted immediately before it (engines execute their stream in order,
    so this is equivalent)."""
    uid = 0
    for _, bbwrap in nc.bb_map.items():
        lst = bbwrap.bb.instructions
        i = 0
        while i < len(lst):
            inst = lst[i]
            si = inst.sync_info
            waits = list(si.on_wait) if si is not None and si.on_wait else []
            if len(waits) > maxw:
                si.on_wait = waits[:maxw]
                for ws in range(maxw, len(waits), maxw):
                    nop = mybir.InstNoOp(name=f"waitsplit-{uid}", ins=[], outs=[])
                    uid += 1
                    nop.engine = inst.engine
                    nop.sync_info = mybir.SyncInfo(
                        on_update=[], on_wait=waits[ws:ws + maxw]
                    )
                    lst.insert(i, nop)
                    i += 1
            i += 1


def _build(mode: str):
    import concourse.bass as bass
    import concourse.mybir as mybir
    import concourse.tile as tile

    f32 = mybir.dt.float32
    f32r = mybir.dt.float32r
    u32 = mybir.dt.uint32

    _TC = tile.TileContext

    nc = bass.Bass("TRN2", target_bir_lowering=False, debug=False, num_devices=NCORES)

    # matmul operand dtype: the BIR verifier requires fp32r operands to be
    # produced as fp32r, so declare them that way end-to-end (same 4 bytes).
    mdt = f32 if mode == "fp32" else f32r

    # x components, (BPC, D, HW) each; for fp22 modes these are the hi/lo parts
    # (DRAM I/O stays f32 -- NEFF I/O rejects f32r -- and is bitcast at the
    # DMA into f32r SBUF tiles, which the host pre-rounded to fp22.)
    xcomp_names = {"fp32": ["x0"], "fp22": ["x0"], "fp22x3": ["x0", "x1"]}[mode]
    xin = {n: nc.dram_tensor(n, [BPC, D, HW], f32, kind="ExternalInput").ap()
           for n in xcomp_names}
    ecomp_names = {"fp32": ["e0"], "fp22": ["e0"], "fp22x3": ["e0", "e1"]}[mode]
    ein = {n: nc.dram_tensor(n, [D, K], f32, kind="ExternalInput").ap()
           for n in ecomp_names}
    # bias rows (-0.5*||e||^2): row 0 = hi, row 1 = lo residual
    nbin = nc.dram_tensor("nb", [2, K], f32, kind="ExternalInput").ap()
    onein = nc.dram_tensor("ones", [2, 128], f32, kind="ExternalInput").ap()
    # gather table (original fp32 codebook, transposed)
    etab = nc.dram_tensor("etab", [K, D], f32, kind="ExternalInput").ap()

    qout = nc.dram_tensor("q", [NPC, D], f32, kind="ExternalOutput").ap()
    iout = nc.dram_tensor("idx", [NPC, 1], u32, kind="ExternalOutput").ap()
    mout = nc.dram_tensor("mx", [NPC, 8], f32, kind="ExternalOutput").ap()

    with _TC(nc) as tc:
        with (
            tc.tile_pool(name="const", bufs=1) as const_pool,
            tc.tile_pool(name="xblk", bufs=2) as x_pool,
            tc.tile_pool(name="scores", bufs=3) as s_pool,
            tc.tile_pool(name="small", bufs=4) as sm_pool,
            tc.tile_pool(name="qrows", bufs=4) as q_pool,
            tc.tile_pool(name="psum", bufs=6, space="PSUM") as p_pool,
            tc.tile_pool(name="warm", bufs=1, space="PSUM") as w_pool,
        ):
            # --- persistent SBUF state -------------------------------------
            emb_sb = {}  # (comp, ktile, codeblock) -> (128, CB) tile
            for n in ecomp_names:
                for kt in range(2):
                    for j in range(NCB):
                        t = const_pool.tile(
                            [128, CB], mdt, tag=f"emb_{n}_{kt}_{j}",
                            name=f"emb_{n}_{kt}_{j}",
                        )
                        nc.sync.dma_start(
                            t[:],
                            ein[n][kt * 128:(kt + 1) * 128,
                                   j * CB:(j + 1) * CB].bitcast(mdt),
                        )
                        emb_sb[(n, kt, j)] = t
            nb_sb = const_pool.tile([2, K], mdt, tag="nb")
            nc.sync.dma_start(nb_sb[:], nbin[:].bitcast(mdt))
            ones_sb = const_pool.tile([2, 128], mdt, tag="ones")
            nc.sync.dma_start(ones_sb[:], onein[:].bitcast(mdt))

            # (lhsT name, rhs name) per data pass; bias uses both nb rows
            if mode == "fp22x3":
                data_passes = [("x0", "e0"), ("x1", "e0"), ("x0", "e1")]
            else:
                data_passes = [("x0", "e0")]

            for b in range(BPC):
                # load this batch's x components: 2 K-tiles x ncomp of (128, HW)
                xb = {}
                for n in xcomp_names:
                    for kt in range(2):
                        t = x_pool.tile([128, HW], mdt, tag=f"xb_{n}_{kt}")
                        nc.sync.dma_start(
                            t[:],
                            xin[n][b, kt * 128:(kt + 1) * 128, :].bitcast(mdt),
                        )
                        xb[(n, kt)] = t

                for c in range(HW // LT):
                    lt = b * (HW // LT) + c
                    sc = s_pool.tile([LT, K], f32, tag="sc")
                    for j in range(NCB):
                        js = slice(j * CB, (j + 1) * CB)
                        # one PSUM bank per code block -- fine-grained so PE
                        # matmuls interleave with ACT copies instead of
                        # bursting and letting HAM re-throttle the PE clock
                        ps = p_pool.tile([LT, CB], f32, tag="ps")
                        # hi+lo bias rows fold into one K=2 matmul:
                        # ones2.T @ [nb_hi; nb_lo] = nb_hi + nb_lo broadcast
                        nc.tensor.matmul(
                            ps[:],
                            lhsT=ones_sb[:],
                            rhs=nb_sb[:, js],
                            start=True, stop=False,
                        )
                        for pi, (xn, en) in enumerate(data_passes):
                            last_pass = pi == len(data_passes) - 1
                            for kt in range(2):
                                nc.tensor.matmul(
                                    ps[:],
                                    lhsT=xb[(xn, kt)][:, c * LT:(c + 1) * LT],
                                    rhs=emb_sb[(en, kt, j)][:],
                                    start=False,
                                    stop=last_pass and kt == 1,
                                )
                        nc.scalar.copy(sc[:, js], ps[:])
                        for _ in range(NDUMMY):
                            wp = w_pool.tile([LT, CB], f32, tag="wp", name="wp")
                            nc.tensor.matmul(
                                wp[:], lhsT=ones_sb[:], rhs=nb_sb[:, 0:CB],
                                start=True, stop=True,
                            )

                    mx = sm_pool.tile([LT, 8], f32, tag="mx")
                    nc.vector.max(mx[:], sc[:])
                    ix = sm_pool.tile([LT, 8], u32, tag="ix")
                    nc.vector.max_index(ix[:], mx[:], sc[:])

                    qr = q_pool.tile([LT, D], f32, tag="qr")
                    nc.gpsimd.indirect_dma_start(
                        out=qr[:],
                        out_offset=None,
                        in_=etab[:],
                        in_offset=bass.IndirectOffsetOnAxis(
                            ap=ix[:, :1], axis=0
                        ),
                    )
                    nc.sync.dma_start(qout[lt * LT:(lt + 1) * LT, :], qr[:])
                    nc.sync.dma_start(iout[lt * LT:(lt + 1) * LT, :], ix[:, :1])
                    nc.sync.dma_start(mout[lt * LT:(lt + 1) * LT, :], mx[:])

    _split_excess_waits(nc, mybir)
    return nc


def _prep_inputs(x: np.ndarray, emb: np.ndarray, mode: str):
    x = np.ascontiguousarray(x, dtype=np.float32)
    emb = np.ascontiguousarray(emb, dtype=np.float32)
    xr = x.reshape(B, D, HW)

    e2 = np.einsum("dk,dk->k", emb, emb, dtype=np.float32).astype(np.float32)
    nb_full = (-0.5 * e2).astype(np.float32)
    nb_hi, nb_lo = _split_hi_lo(nb_full)
    nb = np.stack([nb_hi, nb_lo], axis=0)

    etab = np.ascontiguousarray(emb.T)
    ones = np.ones((2, 128), dtype=np.float32)

    if mode == "fp32":
        shared = {"e0": emb, "nb": nb, "etab": etab, "ones": ones}
        xcomp = {"x0": xr}
    elif mode == "fp22":
        shared = {"e0": _fp22_round(emb), "nb": nb, "etab": etab, "ones": ones}
        xcomp = {"x0": _fp22_round(xr)}
    else:  # fp22x3
        e_hi, e_lo = _split_hi_lo(emb)
        x_hi, x_lo = _split_hi_lo(xr)
        shared = {"e0": e_hi, "e1": e_lo, "nb": nb, "etab": etab, "ones": ones}
        xcomp = {"x0": x_hi, "x1": x_lo}

    in_maps = []
    for c in range(NCORES):
        m = dict(shared)
        for n, arr in xcomp.items():
            m[n] = np.ascontiguousarray(arr[c * BPC:(c + 1) * BPC])
        in_maps.append(m)
    return in_maps


def _run_spmd(in_maps, mode: str, trace: bool = False, tmpdir=None):
    from concourse.bass_utils import run_bass_kernel_spmd

    if mode not in _cache:
        _cache[mode] = _build(mode)
    nc = _cache[mode]
    return run_bass_kernel_spmd(
        nc, in_maps, list(range(NCORES)), trace=trace, tmpdir=tmpdir
    )


def _assemble(res, x: np.ndarray, emb: np.ndarray, mode: str):
    """Gather per-core shards into the full output; in fp22 mode, exactly
    re-resolve rows whose coarse top-2 margin is below TAU."""
    q = np.concatenate([res.results[c]["q"] for c in range(NCORES)], axis=0)
    idx = np.concatenate(
        [res.results[c]["idx"][:, 0] for c in range(NCORES)], axis=0
    ).astype(np.int64)
    n_rescued = 0
    if mode == "fp22":
        mx = np.concatenate(
            [res.results[c]["mx"] for c in range(NCORES)], axis=0
        )
        margin = mx[:, 0] - mx[:, 1]
        sus = np.nonzero(margin < TAU)[0]
        n_rescued = len(sus)
        if n_rescued:
            bi, hw = np.divmod(sus, HW)
            hi, wi = np.divmod(hw, W)
            xs = np.ascontiguousarray(x[bi, :, hi, wi], dtype=np.float32)
            e2 = np.sum(emb.astype(np.float32) ** 2, axis=0, dtype=np.float32)
            d2 = (
                np.sum(xs * xs, axis=1, keepdims=True)
                - 2.0 * (xs @ emb.astype(np.float32))
                + e2[None, :]
            ).astype(np.float32)
            new_idx = np.argmin(d2, axis=1)
            changed = new_idx != idx[sus]
            if changed.any():
                rows = sus[changed]
                idx[rows] = new_idx[changed]
                q[rows] = emb.T[idx[rows]]
    out = q.reshape(B, H, W, D)
    out = np.ascontiguousarray(np.moveaxis(out, -1, 1))
    return out, idx, n_rescued


def kernel(x: np.ndarray, emb: np.ndarray) -> np.ndarray:
    in_maps = _prep_inputs(x, emb, MODE)
    res = _run_spmd(in_maps, MODE)
    out, _, _ = _assemble(res, x, emb, MODE)
    return out


# revision 24
# speedup vs baseline: 1.9014x; 1.9014x over previous
"""VQ codebook nearest-embedding kernel for 8x Trainium2 NeuronCores.

Computes, for x (32, 256, 32, 32) and emb (256, 2048):
    xf = moveaxis(x, 1, -1).reshape(-1, 256)           # (N=32768, 256)
    argmin_n = argmin_k ||xf_n - emb[:, k]||^2
    out[n] = emb[:, argmin_n]   -> reshaped back to (32, 256, 32, 32)

Sharding: data-parallel over the batch dim (4 batches per core, N_c = 4096
latent positions per core); the codebook is replicated.

Key layout fact: x[b] is (D=256, H*W=1024) contiguous, which is exactly the
lhsT (K=D on partitions, M=latents on free) layout the PE wants -- no
transposes anywhere.  argmin_k d2 == argmax_k (x.e_k - 0.5*||e_k||^2); the
bias is folded into the PSUM accumulation group as a K=1 ones-outer-product
matmul, so per 128-latent tile the full (128, 2048) score block lands in
PSUM off pure matmuls.  ScalarE copies scores to SBUF, VectorE max +
max_index extract the argmax, and an indirect DMA gathers the winning
codebook rows.
"""

import os
import sys

import numpy as np

if os.path.isdir("/opt/trn_rl_repo") and "/opt/trn_rl_repo" not in sys.path:
    sys.path.append("/opt/trn_rl_repo")

B, D, H, W = 32, 256, 32, 32
K = 2048
NCORES = 8
BPC = B // NCORES          # batches per core
HW = H * W                 # latents per batch
NPC = BPC * HW             # latents per core
LT = 128                   # latent tile (PSUM partition dim)
NLT = NPC // LT            # latent tiles per core
CB = 512                   # code block (PSUM bank free dim)
NCB = K // CB              # code blocks

# "fp32": native fp32 matmuls (4 cyc/row).
# "fp22x3": hi/lo split operands, 3 float32r passes (~fp32 precision, 3 cyc/row).
# "fp22": single float32r pass (1 cyc/row); coarse scores carry ~1e-3-scale
#   rounding error, so rows whose top-2 score margin falls below TAU are
#   re-argmin'd exactly on the host from the original fp32 data.
MODE = os.environ.get("VQ_KERNEL_MODE", "fp22x3")
TAU = float(os.environ.get("VQ_TAU", "0.05"))
# keep-warm matmuls per code block: PE HAM re-throttles to 1.2 GHz when the
# array's duty cycle drops in DVE-bound phases; dummy N=512 matmuls into a
# scratch PSUM bank keep it at 2.4 GHz.
NDUMMY = int(os.environ.get("VQ_DUMMY", "2"))

_cache = {}


def _fp22_round(a: np.ndarray) -> np.ndarray:
    """Round fp32 mantissa to 11 bits -- the PE float32r operand-read
    precision (HW-probed: RTN-even keeping mantissa bits >= 2^-11; products
    of two such values are exact in the fp32 accumulation)."""
    b = a.astype(np.float32).view(np.uint32).astype(np.uint64)
    r = ((b + 0x800) & 0xFFFFF000).astype(np.uint32)
    return r.view(np.float32)


def _split_hi_lo(a: np.ndarray):
    hi = _fp22_round(a)
    lo = (a.astype(np.float32) - hi).astype(np.float32)
    return hi, lo


def _split_excess_waits(nc, mybir, maxw: int = 1):
    """Walrus's per-instruction encodings have very few sem-wait slots.
    Move all but `maxw` of each instruction's waits onto same-engine NOPs
    inserted immediately before it (engines execute their stream in order,
    so this is equivalent)."""
    uid = 0
    for _, bbwrap in nc.bb_map.items():
        lst = bbwrap.bb.instructions
        i = 0
        while i < len(lst):
            inst = lst[i]
            si = inst.sync_info
            waits = list(si.on_wait) if si is not None and si.on_wait else []
            if len(waits) > maxw:
                si.on_wait = waits[:maxw]
                for ws in range(maxw, len(waits), maxw):
                    nop = mybir.InstNoOp(name=f"waitsplit-{uid}", ins=[], outs=[])
                    uid += 1
                    nop.engine = inst.engine
                    nop.sync_info = mybir.SyncInfo(
                        on_update=[], on_wait=waits[ws:ws + maxw]
                    )
                    lst.insert(i, nop)
                    i += 1
            i += 1


def _build(mode: str):
    import concourse.bass as bass
    import concourse.mybir as mybir
    import concourse.tile as tile

    f32 = mybir.dt.float32
    f32r = mybir.dt.float32r
    u32 = mybir.dt.uint32

    _TC = tile.TileContext

    nc = bass.Bass("TRN2", target_bir_lowering=False, debug=False, num_devices=NCORES)

    # matmul operand dtype: the BIR verifier requires fp32r operands to be
    # produced as fp32r, so declare them that way end-to-end (same 4 bytes).
    mdt = f32 if mode == "fp32" else f32r

    # x components, (BPC, D, HW) each; for fp22 modes these are the hi/lo parts
    # (DRAM I/O stays f32 -- NEFF I/O rejects f32r -- and is bitcast at the
    # DMA into f32r SBUF tiles, which the host pre-rounded to fp22.)
    xcomp_names = {"fp32": ["x0"], "fp22": ["x0"], "fp22x3": ["x0", "x1"]}[mode]
    xin = {n: nc.dram_tensor(n, [BPC, D, HW], f32, kind="ExternalInput").ap()
           for n in xcomp_names}
    ecomp_names = {"fp32": ["e0"], "fp22": ["e0"], "fp22x3": ["e0", "e1"]}[mode]
    ein = {n: nc.dram_tensor(n, [D, K], f32, kind="ExternalInput").ap()
           for n in ecomp_names}
    # bias rows (-0.5*||e||^2): row 0 = hi, row 1 = lo residual
    nbin = nc.dram_tensor("nb", [128, K], f32, kind="ExternalInput").ap()
    onein = nc.dram_tensor("ones", [128, 128], f32, kind="ExternalInput").ap()
    # gather table (original fp32 codebook, transposed)
    etab = nc.dram_tensor("etab", [K, D], f32, kind="ExternalInput").ap()

    qout = nc.dram_tensor("q", [NPC, D], f32, kind="ExternalOutput").ap()
    iout = nc.dram_tensor("idx", [NPC, 1], u32, kind="ExternalOutput").ap()
    mout = nc.dram_tensor("mx", [NPC, 8], f32, kind="ExternalOutput").ap()

    with _TC(nc) as tc:
        with (
            tc.tile_pool(name="const", bufs=1) as const_pool,
            tc.tile_pool(name="xblk", bufs=2) as x_pool,
            tc.tile_pool(name="scores", bufs=3) as s_pool,
            tc.tile_pool(name="small", bufs=4) as sm_pool,
            tc.tile_pool(name="qrows", bufs=4) as q_pool,
            tc.tile_pool(name="psum", bufs=6, space="PSUM") as p_pool,
            tc.tile_pool(name="warm", bufs=1, space="PSUM") as w_pool,
        ):
            # --- persistent SBUF state -------------------------------------
            emb_sb = {}  # (comp, ktile, codeblock) -> (128, CB) tile
            for n in ecomp_names:
                for kt in range(2):
                    for j in range(NCB):
                        t = const_pool.tile(
                            [128, CB], mdt, tag=f"emb_{n}_{kt}_{j}",
                            name=f"emb_{n}_{kt}_{j}",
                        )
                        nc.sync.dma_start(
                            t[:],
                            ein[n][kt * 128:(kt + 1) * 128,
                                   j * CB:(j + 1) * CB].bitcast(mdt),
                        )
                        emb_sb[(n, kt, j)] = t
            nb_sb = const_pool.tile([128, K], mdt, tag="nb")
            nc.sync.dma_start(nb_sb[:], nbin[:].bitcast(mdt))
            ones_sb = const_pool.tile([128, 128], mdt, tag="ones")
            nc.sync.dma_start(ones_sb[:], onein[:].bitcast(mdt))

            # (lhsT name, rhs name) per data pass; bias uses both nb rows
            if mode == "fp22x3":
                data_passes = [("x0", "e0"), ("x1", "e0"), ("x0", "e1")]
            else:
                data_passes = [("x0", "e0")]

            for b in range(BPC):
                # load this batch's x components: 2 K-tiles x ncomp of (128, HW)
                xb = {}
                for n in xcomp_names:
                    for kt in range(2):
                        t = x_pool.tile([128, HW], mdt, tag=f"xb_{n}_{kt}")
                        nc.sync.dma_start(
                            t[:],
                            xin[n][b, kt * 128:(kt + 1) * 128, :].bitcast(mdt),
                        )
                        xb[(n, kt)] = t

                for c in range(HW // LT):
                    lt = b * (HW // LT) + c
                    sc = s_pool.tile([LT, K], f32, tag="sc")
                    for j in range(NCB):
                        js = slice(j * CB, (j + 1) * CB)
                        # one PSUM bank per code block -- fine-grained so PE
                        # matmuls interleave with ACT copies instead of
                        # bursting and letting HAM re-throttle the PE clock
                        ps = p_pool.tile([LT, CB], f32, tag="ps")
                        # bias: ones128.T @ [nb_hi; nb_lo; 0...] -- K=128 so
                        # the PE array runs fully utilized (HAM stays warm)
                        nc.tensor.matmul(
                            ps[:],
                            lhsT=ones_sb[:],
                            rhs=nb_sb[:, js],
                            start=True, stop=False,
                        )
                        for pi, (xn, en) in enumerate(data_passes):
                            last_pass = pi == len(data_passes) - 1
                            for kt in range(2):
                                nc.tensor.matmul(
                                    ps[:],
                                    lhsT=xb[(xn, kt)][:, c * LT:(c + 1) * LT],
                                    rhs=emb_sb[(en, kt, j)][:],
                                    start=False,
                                    stop=last_pass and kt == 1,
                                )
                        nc.scalar.copy(sc[:, js], ps[:])
                        for _ in range(NDUMMY):
                            wp = w_pool.tile([LT, CB], f32, tag="wp", name="wp")
                            nc.tensor.matmul(
                                wp[:], lhsT=ones_sb[:], rhs=nb_sb[:, 0:CB],
                                start=True, stop=True,
                            )

                    mx = sm_pool.tile([LT, 8], f32, tag="mx")
                    nc.vector.max(mx[:], sc[:])
                    ix = sm_pool.tile([LT, 8], u32, tag="ix")
                    nc.vector.max_index(ix[:], mx[:], sc[:])

                    qr = q_pool.tile([LT, D], f32, tag="qr")
                    nc.gpsimd.indirect_dma_start(
                        out=qr[:],
                        out_offset=None,
                        in_=etab[:],
                        in_offset=bass.IndirectOffsetOnAxis(
                            ap=ix[:, :1], axis=0
                        ),
                    )
                    nc.sync.dma_start(qout[lt * LT:(lt + 1) * LT, :], qr[:])
                    nc.sync.dma_start(iout[lt * LT:(lt + 1) * LT, :], ix[:, :1])
                    nc.sync.dma_start(mout[lt * LT:(lt + 1) * LT, :], mx[:])

    _split_excess_waits(nc, mybir)
    return nc


def _prep_inputs(x: np.ndarray, emb: np.ndarray, mode: str):
    x = np.ascontiguousarray(x, dtype=np.float32)
    emb = np.ascontiguousarray(emb, dtype=np.float32)
    xr = x.reshape(B, D, HW)

    e2 = np.einsum("dk,dk->k", emb, emb, dtype=np.float32).astype(np.float32)
    nb_full = (-0.5 * e2).astype(np.float32)
    nb_hi, nb_lo = _split_hi_lo(nb_full)
    nb = np.zeros((128, K), dtype=np.float32)
    nb[0] = nb_hi
    nb[1] = nb_lo

    etab = np.ascontiguousarray(emb.T)
    ones = np.ones((128, 128), dtype=np.float32)

    if mode == "fp32":
        shared = {"e0": emb, "nb": nb, "etab": etab, "ones": ones}
        xcomp = {"x0": xr}
    elif mode == "fp22":
        shared = {"e0": _fp22_round(emb), "nb": nb, "etab": etab, "ones": ones}
        xcomp = {"x0": _fp22_round(xr)}
    else:  # fp22x3
        e_hi, e_lo = _split_hi_lo(emb)
        x_hi, x_lo = _split_hi_lo(xr)
        shared = {"e0": e_hi, "e1": e_lo, "nb": nb, "etab": etab, "ones": ones}
        xcomp = {"x0": x_hi, "x1": x_lo}

    in_maps = []
    for c in range(NCORES):
        m = dict(shared)
        for n, arr in xcomp.items():
            m[n] = np.ascontiguousarray(arr[c * BPC:(c + 1) * BPC])
        in_maps.append(m)
    return in_maps


def _run_spmd(in_maps, mode: str, trace: bool = False, tmpdir=None):
    from concourse.bass_utils import run_bass_kernel_spmd

    if mode not in _cache:
        _cache[mode] = _build(mode)
    nc = _cache[mode]
    return run_bass_kernel_spmd(
        nc, in_maps, list(range(NCORES)), trace=trace, tmpdir=tmpdir
    )


def _assemble(res, x: np.ndarray, emb: np.ndarray, mode: str):
    """Gather per-core shards into the full output; in fp22 mode, exactly
    re-resolve rows whose coarse top-2 margin is below TAU."""
    q = np.concatenate([res.results[c]["q"] for c in range(NCORES)], axis=0)
    idx = np.concatenate(
        [res.results[c]["idx"][:, 0] for c in range(NCORES)], axis=0
    ).astype(np.int64)
    n_rescued = 0
    if mode == "fp22":
        mx = np.concatenate(
            [res.results[c]["mx"] for c in range(NCORES)], axis=0
        )
        margin = mx[:, 0] - mx[:, 1]
        sus = np.nonzero(margin < TAU)[0]
        n_rescued = len(sus)
        if n_rescued:
            bi, hw = np.divmod(sus, HW)
            hi, wi = np.divmod(hw, W)
            xs = np.ascontiguousarray(x[bi, :, hi, wi], dtype=np.float32)
            e2 = np.sum(emb.astype(np.float32) ** 2, axis=0, dtype=np.float32)
            d2 = (
                np.sum(xs * xs, axis=1, keepdims=True)
                - 2.0 * (xs @ emb.astype(np.float32))
                + e2[None, :]
            ).astype(np.float32)
            new_idx = np.argmin(d2, axis=1)
            changed = new_idx != idx[sus]
            if changed.any():
                rows = sus[changed]
                idx[rows] = new_idx[changed]
                q[rows] = emb.T[idx[rows]]
    out = q.reshape(B, H, W, D)
    out = np.ascontiguousarray(np.moveaxis(out, -1, 1))
    return out, idx, n_rescued


def kernel(x: np.ndarray, emb: np.ndarray) -> np.ndarray:
    in_maps = _prep_inputs(x, emb, MODE)
    res = _run_spmd(in_maps, MODE)
    out, _, _ = _assemble(res, x, emb, MODE)
    return out


# revision 25
# speedup vs baseline: 1.9784x; 1.0405x over previous
"""VQ codebook nearest-embedding kernel for 8x Trainium2 NeuronCores.

Computes, for x (32, 256, 32, 32) and emb (256, 2048):
    xf = moveaxis(x, 1, -1).reshape(-1, 256)           # (N=32768, 256)
    argmin_n = argmin_k ||xf_n - emb[:, k]||^2
    out[n] = emb[:, argmin_n]   -> reshaped back to (32, 256, 32, 32)

Sharding: data-parallel over the batch dim (4 batches per core, N_c = 4096
latent positions per core); the codebook is replicated.

Key layout fact: x[b] is (D=256, H*W=1024) contiguous, which is exactly the
lhsT (K=D on partitions, M=latents on free) layout the PE wants -- no
transposes anywhere.  argmin_k d2 == argmax_k (x.e_k - 0.5*||e_k||^2); the
bias is folded into the PSUM accumulation group as a K=1 ones-outer-product
matmul, so per 128-latent tile the full (128, 2048) score block lands in
PSUM off pure matmuls.  ScalarE copies scores to SBUF, VectorE max +
max_index extract the argmax, and an indirect DMA gathers the winning
codebook rows.
"""

import os
import sys

import numpy as np

if os.path.isdir("/opt/trn_rl_repo") and "/opt/trn_rl_repo" not in sys.path:
    sys.path.append("/opt/trn_rl_repo")

B, D, H, W = 32, 256, 32, 32
K = 2048
NCORES = 8
BPC = B // NCORES          # batches per core
HW = H * W                 # latents per batch
NPC = BPC * HW             # latents per core
LT = 128                   # latent tile (PSUM partition dim)
NLT = NPC // LT            # latent tiles per core
CB = 512                   # code block (PSUM bank free dim)
NCB = K // CB              # code blocks

# "fp32": native fp32 matmuls (4 cyc/row).
# "fp22x3": hi/lo split operands, 3 float32r passes (~fp32 precision, 3 cyc/row).
# "fp22": single float32r pass (1 cyc/row); coarse scores carry ~1e-3-scale
#   rounding error, so rows whose top-2 score margin falls below TAU are
#   re-argmin'd exactly on the host from the original fp32 data.
MODE = os.environ.get("VQ_KERNEL_MODE", "fp22x3")
TAU = float(os.environ.get("VQ_TAU", "0.05"))
# keep-warm matmuls per code block: PE HAM re-throttles to 1.2 GHz when the
# array's duty cycle drops in DVE-bound phases; dummy N=512 matmuls into a
# scratch PSUM bank keep it at 2.4 GHz.
NDUMMY = int(os.environ.get("VQ_DUMMY", "2"))

_cache = {}


def _fp22_round(a: np.ndarray) -> np.ndarray:
    """Round fp32 mantissa to 11 bits -- the PE float32r operand-read
    precision (HW-probed: RTN-even keeping mantissa bits >= 2^-11; products
    of two such values are exact in the fp32 accumulation)."""
    b = a.astype(np.float32).view(np.uint32).astype(np.uint64)
    r = ((b + 0x800) & 0xFFFFF000).astype(np.uint32)
    return r.view(np.float32)


def _split_hi_lo(a: np.ndarray):
    hi = _fp22_round(a)
    lo = (a.astype(np.float32) - hi).astype(np.float32)
    return hi, lo


def _split_excess_waits(nc, mybir, maxw: int = 1):
    """Walrus's per-instruction encodings have very few sem-wait slots.
    Move all but `maxw` of each instruction's waits onto same-engine NOPs
    inserted immediately before it (engines execute their stream in order,
    so this is equivalent)."""
    uid = 0
    for _, bbwrap in nc.bb_map.items():
        lst = bbwrap.bb.instructions
        i = 0
        while i < len(lst):
            inst = lst[i]
            si = inst.sync_info
            waits = list(si.on_wait) if si is not None and si.on_wait else []
            if len(waits) > maxw:
                si.on_wait = waits[:maxw]
                for ws in range(maxw, len(waits), maxw):
                    nop = mybir.InstNoOp(name=f"waitsplit-{uid}", ins=[], outs=[])
                    uid += 1
                    nop.engine = inst.engine
                    nop.sync_info = mybir.SyncInfo(
                        on_update=[], on_wait=waits[ws:ws + maxw]
                    )
                    lst.insert(i, nop)
                    i += 1
            i += 1


def _build(mode: str):
    import concourse.bass as bass
    import concourse.mybir as mybir
    import concourse.tile as tile

    f32 = mybir.dt.float32
    f32r = mybir.dt.float32r
    u32 = mybir.dt.uint32

    _TC = tile.TileContext

    nc = bass.Bass("TRN2", target_bir_lowering=False, debug=False, num_devices=NCORES)

    # matmul operand dtype: the BIR verifier requires fp32r operands to be
    # produced as fp32r, so declare them that way end-to-end (same 4 bytes).
    mdt = f32 if mode == "fp32" else f32r

    # x components, (BPC, D, HW) each; for fp22 modes these are the hi/lo parts
    # (DRAM I/O stays f32 -- NEFF I/O rejects f32r -- and is bitcast at the
    # DMA into f32r SBUF tiles, which the host pre-rounded to fp22.)
    xcomp_names = {"fp32": ["x0"], "fp22": ["x0"], "fp22x3": ["x0", "x1"]}[mode]
    xin = {n: nc.dram_tensor(n, [BPC, D, HW], f32, kind="ExternalInput").ap()
           for n in xcomp_names}
    ecomp_names = {"fp32": ["e0"], "fp22": ["e0"], "fp22x3": ["e0", "e1"]}[mode]
    ein = {n: nc.dram_tensor(n, [D, K], f32, kind="ExternalInput").ap()
           for n in ecomp_names}
    # bias rows (-0.5*||e||^2): row 0 = hi, row 1 = lo residual
    nbin = nc.dram_tensor("nb", [128, K], f32, kind="ExternalInput").ap()
    onein = nc.dram_tensor("ones", [128, 128], f32, kind="ExternalInput").ap()
    # gather table (original fp32 codebook, transposed)
    etab = nc.dram_tensor("etab", [K, D], f32, kind="ExternalInput").ap()

    qout = nc.dram_tensor("q", [NPC, D], f32, kind="ExternalOutput").ap()
    iout = nc.dram_tensor("idx", [NPC, 1], u32, kind="ExternalOutput").ap()
    mout = nc.dram_tensor("mx", [NPC, 8], f32, kind="ExternalOutput").ap()

    with _TC(nc) as tc:
        with (
            tc.tile_pool(name="const", bufs=1) as const_pool,
            tc.tile_pool(name="xblk", bufs=2) as x_pool,
            tc.tile_pool(name="scores", bufs=4) as s_pool,
            tc.tile_pool(name="small", bufs=4) as sm_pool,
            tc.tile_pool(name="qrows", bufs=4) as q_pool,
            tc.tile_pool(name="psum", bufs=6, space="PSUM") as p_pool,
            tc.tile_pool(name="warm", bufs=1, space="PSUM") as w_pool,
        ):
            # --- persistent SBUF state -------------------------------------
            # DMA emission order = rough fetch priority: the j=0 codebook
            # chunks + bias + ones go first so the first matmul group can
            # start while the rest of the codebook streams in.
            ones_sb = const_pool.tile([128, 128], mdt, tag="ones")
            nc.sync.dma_start(ones_sb[:], onein[:].bitcast(mdt))
            nb_sb = const_pool.tile([128, K], mdt, tag="nb")
            nc.sync.dma_start(nb_sb[:], nbin[:].bitcast(mdt))

            emb_sb = {}  # (comp, ktile, codeblock) -> (128, CB) tile

            def _load_emb_chunks(jlist):
                for n in ecomp_names:
                    for kt in range(2):
                        for j in jlist:
                            t = const_pool.tile(
                                [128, CB], mdt, tag=f"emb_{n}_{kt}_{j}",
                                name=f"emb_{n}_{kt}_{j}",
                            )
                            nc.sync.dma_start(
                                t[:],
                                ein[n][kt * 128:(kt + 1) * 128,
                                       j * CB:(j + 1) * CB].bitcast(mdt),
                            )
                            emb_sb[(n, kt, j)] = t

            _load_emb_chunks([0])

            # (lhsT name, rhs name) per data pass; bias uses both nb rows
            if mode == "fp22x3":
                data_passes = [("x0", "e0"), ("x1", "e0"), ("x0", "e1")]
            else:
                data_passes = [("x0", "e0")]

            for b in range(BPC):
                # load this batch's x components: 2 K-tiles x ncomp of (128, HW)
                xb = {}
                for n in xcomp_names:
                    for kt in range(2):
                        t = x_pool.tile([128, HW], mdt, tag=f"xb_{n}_{kt}")
                        nc.sync.dma_start(
                            t[:],
                            xin[n][b, kt * 128:(kt + 1) * 128, :].bitcast(mdt),
                        )
                        xb[(n, kt)] = t
                if b == 0:
                    _load_emb_chunks([1, 2, 3])

                for c in range(HW // LT):
                    lt = b * (HW // LT) + c
                    sc = s_pool.tile([LT, K], f32, tag="sc")
                    for j in range(NCB):
                        js = slice(j * CB, (j + 1) * CB)
                        # one PSUM bank per code block -- fine-grained so PE
                        # matmuls interleave with ACT copies instead of
                        # bursting and letting HAM re-throttle the PE clock
                        ps = p_pool.tile([LT, CB], f32, tag="ps")
                        # bias: ones128.T @ [nb_hi; nb_lo; 0...] -- K=128 so
                        # the PE array runs fully utilized (HAM stays warm)
                        nc.tensor.matmul(
                            ps[:],
                            lhsT=ones_sb[:],
                            rhs=nb_sb[:, js],
                            start=True, stop=False,
                        )
                        for pi, (xn, en) in enumerate(data_passes):
                            last_pass = pi == len(data_passes) - 1
                            for kt in range(2):
                                nc.tensor.matmul(
                                    ps[:],
                                    lhsT=xb[(xn, kt)][:, c * LT:(c + 1) * LT],
                                    rhs=emb_sb[(en, kt, j)][:],
                                    start=False,
                                    stop=last_pass and kt == 1,
                                )
                        nc.scalar.copy(sc[:, js], ps[:])
                        for _ in range(NDUMMY):
                            wp = w_pool.tile([LT, CB], f32, tag="wp", name="wp")
                            nc.tensor.matmul(
                                wp[:], lhsT=ones_sb[:], rhs=nb_sb[:, 0:CB],
                                start=True, stop=True,
                            )

                    mx = sm_pool.tile([LT, 8], f32, tag="mx")
                    nc.vector.max(mx[:], sc[:])
                    ix = sm_pool.tile([LT, 8], u32, tag="ix")
                    nc.vector.max_index(ix[:], mx[:], sc[:])

                    qr = q_pool.tile([LT, D], f32, tag="qr")
                    nc.gpsimd.indirect_dma_start(
                        out=qr[:],
                        out_offset=None,
                        in_=etab[:],
                        in_offset=bass.IndirectOffsetOnAxis(
                            ap=ix[:, :1], axis=0
                        ),
                    )
                    nc.sync.dma_start(qout[lt * LT:(lt + 1) * LT, :], qr[:])
                    nc.sync.dma_start(iout[lt * LT:(lt + 1) * LT, :], ix[:, :1])
                    nc.sync.dma_start(mout[lt * LT:(lt + 1) * LT, :], mx[:])

    _split_excess_waits(nc, mybir)
    return nc


def _prep_inputs(x: np.ndarray, emb: np.ndarray, mode: str):
    x = np.ascontiguousarray(x, dtype=np.float32)
    emb = np.ascontiguousarray(emb, dtype=np.float32)
    xr = x.reshape(B, D, HW)

    e2 = np.einsum("dk,dk->k", emb, emb, dtype=np.float32).astype(np.float32)
    nb_full = (-0.5 * e2).astype(np.float32)
    nb_hi, nb_lo = _split_hi_lo(nb_full)
    nb = np.zeros((128, K), dtype=np.float32)
    nb[0] = nb_hi
    nb[1] = nb_lo

    etab = np.ascontiguousarray(emb.T)
    ones = np.ones((128, 128), dtype=np.float32)

    if mode == "fp32":
        shared = {"e0": emb, "nb": nb, "etab": etab, "ones": ones}
        xcomp = {"x0": xr}
    elif mode == "fp22":
        shared = {"e0": _fp22_round(emb), "nb": nb, "etab": etab, "ones": ones}
        xcomp = {"x0": _fp22_round(xr)}
    else:  # fp22x3
        e_hi, e_lo = _split_hi_lo(emb)
        x_hi, x_lo = _split_hi_lo(xr)
        shared = {"e0": e_hi, "e1": e_lo, "nb": nb, "etab": etab, "ones": ones}
        xcomp = {"x0": x_hi, "x1": x_lo}

    in_maps = []
    for c in range(NCORES):
        m = dict(shared)
        for n, arr in xcomp.items():
            m[n] = np.ascontiguousarray(arr[c * BPC:(c + 1) * BPC])
        in_maps.append(m)
    return in_maps


def _run_spmd(in_maps, mode: str, trace: bool = False, tmpdir=None):
    from concourse.bass_utils import run_bass_kernel_spmd

    if mode not in _cache:
        _cache[mode] = _build(mode)
    nc = _cache[mode]
    return run_bass_kernel_spmd(
        nc, in_maps, list(range(NCORES)), trace=trace, tmpdir=tmpdir
    )


def _assemble(res, x: np.ndarray, emb: np.ndarray, mode: str):
    """Gather per-core shards into the full output; in fp22 mode, exactly
    re-resolve rows whose coarse top-2 margin is below TAU."""
    q = np.concatenate([res.results[c]["q"] for c in range(NCORES)], axis=0)
    idx = np.concatenate(
        [res.results[c]["idx"][:, 0] for c in range(NCORES)], axis=0
    ).astype(np.int64)
    n_rescued = 0
    if mode == "fp22":
        mx = np.concatenate(
            [res.results[c]["mx"] for c in range(NCORES)], axis=0
        )
        margin = mx[:, 0] - mx[:, 1]
        sus = np.nonzero(margin < TAU)[0]
        n_rescued = len(sus)
        if n_rescued:
            bi, hw = np.divmod(sus, HW)
            hi, wi = np.divmod(hw, W)
            xs = np.ascontiguousarray(x[bi, :, hi, wi], dtype=np.float32)
            e2 = np.sum(emb.astype(np.float32) ** 2, axis=0, dtype=np.float32)
            d2 = (
                np.sum(xs * xs, axis=1, keepdims=True)
                - 2.0 * (xs @ emb.astype(np.float32))
                + e2[None, :]
            ).astype(np.float32)
            new_idx = np.argmin(d2, axis=1)
            changed = new_idx != idx[sus]
            if changed.any():
                rows = sus[changed]
                idx[rows] = new_idx[changed]
                q[rows] = emb.T[idx[rows]]
    out = q.reshape(B, H, W, D)
    out = np.ascontiguousarray(np.moveaxis(out, -1, 1))
    return out, idx, n_rescued


def kernel(x: np.ndarray, emb: np.ndarray) -> np.ndarray:
    in_maps = _prep_inputs(x, emb, MODE)
    res = _run_spmd(in_maps, MODE)
    out, _, _ = _assemble(res, x, emb, MODE)
    return out


# revision 26
# speedup vs baseline: 2.0071x; 1.0145x over previous
"""VQ codebook nearest-embedding kernel for 8x Trainium2 NeuronCores.

Computes, for x (32, 256, 32, 32) and emb (256, 2048):
    xf = moveaxis(x, 1, -1).reshape(-1, 256)           # (N=32768, 256)
    argmin_n = argmin_k ||xf_n - emb[:, k]||^2
    out[n] = emb[:, argmin_n]   -> reshaped back to (32, 256, 32, 32)

Sharding: data-parallel over the batch dim (4 batches per core, N_c = 4096
latent positions per core); the codebook is replicated.

Key layout fact: x[b] is (D=256, H*W=1024) contiguous, which is exactly the
lhsT (K=D on partitions, M=latents on free) layout the PE wants -- no
transposes anywhere.  argmin_k d2 == argmax_k (x.e_k - 0.5*||e_k||^2); the
bias is folded into the PSUM accumulation group as a K=1 ones-outer-product
matmul, so per 128-latent tile the full (128, 2048) score block lands in
PSUM off pure matmuls.  ScalarE copies scores to SBUF, VectorE max +
max_index extract the argmax, and an indirect DMA gathers the winning
codebook rows.
"""

import os
import sys

import numpy as np

if os.path.isdir("/opt/trn_rl_repo") and "/opt/trn_rl_repo" not in sys.path:
    sys.path.append("/opt/trn_rl_repo")

B, D, H, W = 32, 256, 32, 32
K = 2048
NCORES = 8
BPC = B // NCORES          # batches per core
HW = H * W                 # latents per batch
NPC = BPC * HW             # latents per core
LT = 128                   # latent tile (PSUM partition dim)
NLT = NPC // LT            # latent tiles per core
CB = 512                   # code block (PSUM bank free dim)
NCB = K // CB              # code blocks

# "fp32": native fp32 matmuls (4 cyc/row).
# "fp22x3": hi/lo split operands, 3 float32r passes (~fp32 precision, 3 cyc/row).
# "fp22": single float32r pass (1 cyc/row); coarse scores carry ~1e-3-scale
#   rounding error, so rows whose top-2 score margin falls below TAU are
#   re-argmin'd exactly on the host from the original fp32 data.
MODE = os.environ.get("VQ_KERNEL_MODE", "fp22x3")
TAU = float(os.environ.get("VQ_TAU", "0.05"))
# keep-warm matmuls per code block: PE HAM re-throttles to 1.2 GHz when the
# array's duty cycle drops in DVE-bound phases; dummy N=512 matmuls into a
# scratch PSUM bank keep it at 2.4 GHz.
NDUMMY = int(os.environ.get("VQ_DUMMY", "2"))

_cache = {}


def _fp22_round(a: np.ndarray) -> np.ndarray:
    """Round fp32 mantissa to 11 bits -- the PE float32r operand-read
    precision (HW-probed: RTN-even keeping mantissa bits >= 2^-11; products
    of two such values are exact in the fp32 accumulation)."""
    b = a.astype(np.float32).view(np.uint32).astype(np.uint64)
    r = ((b + 0x800) & 0xFFFFF000).astype(np.uint32)
    return r.view(np.float32)


def _split_hi_lo(a: np.ndarray):
    hi = _fp22_round(a)
    lo = (a.astype(np.float32) - hi).astype(np.float32)
    return hi, lo


def _split_excess_waits(nc, mybir, maxw: int = 1):
    """Walrus's per-instruction encodings have very few sem-wait slots.
    Move all but `maxw` of each instruction's waits onto same-engine NOPs
    inserted immediately before it (engines execute their stream in order,
    so this is equivalent)."""
    uid = 0
    for _, bbwrap in nc.bb_map.items():
        lst = bbwrap.bb.instructions
        i = 0
        while i < len(lst):
            inst = lst[i]
            si = inst.sync_info
            waits = list(si.on_wait) if si is not None and si.on_wait else []
            if len(waits) > maxw:
                si.on_wait = waits[:maxw]
                for ws in range(maxw, len(waits), maxw):
                    nop = mybir.InstNoOp(name=f"waitsplit-{uid}", ins=[], outs=[])
                    uid += 1
                    nop.engine = inst.engine
                    nop.sync_info = mybir.SyncInfo(
                        on_update=[], on_wait=waits[ws:ws + maxw]
                    )
                    lst.insert(i, nop)
                    i += 1
            i += 1


def _build(mode: str):
    import concourse.bass as bass
    import concourse.mybir as mybir
    import concourse.tile as tile

    f32 = mybir.dt.float32
    f32r = mybir.dt.float32r
    u32 = mybir.dt.uint32

    from concourse.vector_clock import ScopedClock

    class _TC(tile.TileContext):
        def _drain_and_barrier(self, tick_clock, wait_clock):
            nc_ = self.nc
            drain_inst = nc_.sync.drain()
            wait_clock.add_sem_waits(
                drain_inst.ins, ScopedClock({None: tick_clock.global_clock})
            )
            assert self.sems is not None
            popped = nc_._tile_sem_poison_stack.pop()
            assert popped is self._sem_poison

    nc = bass.Bass("TRN2", target_bir_lowering=False, debug=False, num_devices=NCORES)

    # matmul operand dtype: the BIR verifier requires fp32r operands to be
    # produced as fp32r, so declare them that way end-to-end (same 4 bytes).
    mdt = f32 if mode == "fp32" else f32r

    # x components, (BPC, D, HW) each; for fp22 modes these are the hi/lo parts
    # (DRAM I/O stays f32 -- NEFF I/O rejects f32r -- and is bitcast at the
    # DMA into f32r SBUF tiles, which the host pre-rounded to fp22.)
    xcomp_names = {"fp32": ["x0"], "fp22": ["x0"], "fp22x3": ["x0", "x1"]}[mode]
    xin = {n: nc.dram_tensor(n, [BPC, D, HW], f32, kind="ExternalInput").ap()
           for n in xcomp_names}
    ecomp_names = {"fp32": ["e0"], "fp22": ["e0"], "fp22x3": ["e0", "e1"]}[mode]
    ein = {n: nc.dram_tensor(n, [D, K], f32, kind="ExternalInput").ap()
           for n in ecomp_names}
    # bias rows (-0.5*||e||^2): row 0 = hi, row 1 = lo residual
    nbin = nc.dram_tensor("nb", [128, K], f32, kind="ExternalInput").ap()
    onein = nc.dram_tensor("ones", [128, 128], f32, kind="ExternalInput").ap()
    # gather table (original fp32 codebook, transposed)
    etab = nc.dram_tensor("etab", [K, D], f32, kind="ExternalInput").ap()

    qout = nc.dram_tensor("q", [NPC, D], f32, kind="ExternalOutput").ap()
    iout = nc.dram_tensor("idx", [NPC, 1], u32, kind="ExternalOutput").ap()
    mout = nc.dram_tensor("mx", [NPC, 8], f32, kind="ExternalOutput").ap()

    with _TC(nc) as tc:
        with (
            tc.tile_pool(name="const", bufs=1) as const_pool,
            tc.tile_pool(name="xblk", bufs=2) as x_pool,
            tc.tile_pool(name="scores", bufs=4) as s_pool,
            tc.tile_pool(name="small", bufs=4) as sm_pool,
            tc.tile_pool(name="qrows", bufs=4) as q_pool,
            tc.tile_pool(name="psum", bufs=6, space="PSUM") as p_pool,
            tc.tile_pool(name="warm", bufs=1, space="PSUM") as w_pool,
        ):
            # --- persistent SBUF state -------------------------------------
            # DMA emission order = rough fetch priority: the j=0 codebook
            # chunks + bias + ones go first so the first matmul group can
            # start while the rest of the codebook streams in.
            ones_sb = const_pool.tile([128, 128], mdt, tag="ones")
            nc.sync.dma_start(ones_sb[:], onein[:].bitcast(mdt))
            nb_sb = const_pool.tile([128, K], mdt, tag="nb")
            nc.sync.dma_start(nb_sb[:], nbin[:].bitcast(mdt))

            emb_sb = {}  # (comp, ktile, codeblock) -> (128, CB) tile

            def _load_emb_chunks(jlist):
                for n in ecomp_names:
                    for kt in range(2):
                        for j in jlist:
                            t = const_pool.tile(
                                [128, CB], mdt, tag=f"emb_{n}_{kt}_{j}",
                                name=f"emb_{n}_{kt}_{j}",
                            )
                            nc.sync.dma_start(
                                t[:],
                                ein[n][kt * 128:(kt + 1) * 128,
                                       j * CB:(j + 1) * CB].bitcast(mdt),
                            )
                            emb_sb[(n, kt, j)] = t

            _load_emb_chunks([0])

            for _ in range(12):
                wp = w_pool.tile([LT, CB], f32, tag="wp", name="wp")
                nc.tensor.matmul(
                    wp[:], lhsT=ones_sb[:], rhs=nb_sb[:, 0:CB],
                    start=True, stop=True,
                )

            # (lhsT name, rhs name) per data pass; bias uses both nb rows
            if mode == "fp22x3":
                data_passes = [("x0", "e0"), ("x1", "e0"), ("x0", "e1")]
            else:
                data_passes = [("x0", "e0")]

            for b in range(BPC):
                # load this batch's x components: 2 K-tiles x ncomp of (128, HW)
                xb = {}
                for n in xcomp_names:
                    for kt in range(2):
                        t = x_pool.tile([128, HW], mdt, tag=f"xb_{n}_{kt}")
                        nc.sync.dma_start(
                            t[:],
                            xin[n][b, kt * 128:(kt + 1) * 128, :].bitcast(mdt),
                        )
                        xb[(n, kt)] = t
                if b == 0:
                    _load_emb_chunks([1, 2, 3])

                for c in range(HW // LT):
                    lt = b * (HW // LT) + c
                    sc = s_pool.tile([LT, K], f32, tag="sc")
                    for j in range(NCB):
                        js = slice(j * CB, (j + 1) * CB)
                        # one PSUM bank per code block -- fine-grained so PE
                        # matmuls interleave with ACT copies instead of
                        # bursting and letting HAM re-throttle the PE clock
                        ps = p_pool.tile([LT, CB], f32, tag="ps")
                        # bias: ones128.T @ [nb_hi; nb_lo; 0...] -- K=128 so
                        # the PE array runs fully utilized (HAM stays warm)
                        nc.tensor.matmul(
                            ps[:],
                            lhsT=ones_sb[:],
                            rhs=nb_sb[:, js],
                            start=True, stop=False,
                        )
                        for pi, (xn, en) in enumerate(data_passes):
                            last_pass = pi == len(data_passes) - 1
                            for kt in range(2):
                                nc.tensor.matmul(
                                    ps[:],
                                    lhsT=xb[(xn, kt)][:, c * LT:(c + 1) * LT],
                                    rhs=emb_sb[(en, kt, j)][:],
                                    start=False,
                                    stop=last_pass and kt == 1,
                                )
                        nc.scalar.copy(sc[:, js], ps[:])
                        for _ in range(NDUMMY):
                            wp = w_pool.tile([LT, CB], f32, tag="wp", name="wp")
                            nc.tensor.matmul(
                                wp[:], lhsT=ones_sb[:], rhs=nb_sb[:, 0:CB],
                                start=True, stop=True,
                            )

                    mx = sm_pool.tile([LT, 8], f32, tag="mx")
                    nc.vector.max(mx[:], sc[:])
                    ix = sm_pool.tile([LT, 8], u32, tag="ix")
                    nc.vector.max_index(ix[:], mx[:], sc[:])

                    qr = q_pool.tile([LT, D], f32, tag="qr")
                    nc.gpsimd.indirect_dma_start(
                        out=qr[:],
                        out_offset=None,
                        in_=etab[:],
                        in_offset=bass.IndirectOffsetOnAxis(
                            ap=ix[:, :1], axis=0
                        ),
                    )
                    nc.sync.dma_start(qout[lt * LT:(lt + 1) * LT, :], qr[:])
                    nc.sync.dma_start(iout[lt * LT:(lt + 1) * LT, :], ix[:, :1])
                    nc.sync.dma_start(mout[lt * LT:(lt + 1) * LT, :], mx[:])

    _split_excess_waits(nc, mybir)
    return nc


def _prep_inputs(x: np.ndarray, emb: np.ndarray, mode: str):
    x = np.ascontiguousarray(x, dtype=np.float32)
    emb = np.ascontiguousarray(emb, dtype=np.float32)
    xr = x.reshape(B, D, HW)

    e2 = np.einsum("dk,dk->k", emb, emb, dtype=np.float32).astype(np.float32)
    nb_full = (-0.5 * e2).astype(np.float32)
    nb_hi, nb_lo = _split_hi_lo(nb_full)
    nb = np.zeros((128, K), dtype=np.float32)
    nb[0] = nb_hi
    nb[1] = nb_lo

    etab = np.ascontiguousarray(emb.T)
    ones = np.ones((128, 128), dtype=np.float32)

    if mode == "fp32":
        shared = {"e0": emb, "nb": nb, "etab": etab, "ones": ones}
        xcomp = {"x0": xr}
    elif mode == "fp22":
        shared = {"e0": _fp22_round(emb), "nb": nb, "etab": etab, "ones": ones}
        xcomp = {"x0": _fp22_round(xr)}
    else:  # fp22x3
        e_hi, e_lo = _split_hi_lo(emb)
        x_hi, x_lo = _split_hi_lo(xr)
        shared = {"e0": e_hi, "e1": e_lo, "nb": nb, "etab": etab, "ones": ones}
        xcomp = {"x0": x_hi, "x1": x_lo}

    in_maps = []
    for c in range(NCORES):
        m = dict(shared)
        for n, arr in xcomp.items():
            m[n] = np.ascontiguousarray(arr[c * BPC:(c + 1) * BPC])
        in_maps.append(m)
    return in_maps


def _run_spmd(in_maps, mode: str, trace: bool = False, tmpdir=None):
    from concourse.bass_utils import run_bass_kernel_spmd

    if mode not in _cache:
        _cache[mode] = _build(mode)
    nc = _cache[mode]
    return run_bass_kernel_spmd(
        nc, in_maps, list(range(NCORES)), trace=trace, tmpdir=tmpdir
    )


def _assemble(res, x: np.ndarray, emb: np.ndarray, mode: str):
    """Gather per-core shards into the full output; in fp22 mode, exactly
    re-resolve rows whose coarse top-2 margin is below TAU."""
    q = np.concatenate([res.results[c]["q"] for c in range(NCORES)], axis=0)
    idx = np.concatenate(
        [res.results[c]["idx"][:, 0] for c in range(NCORES)], axis=0
    ).astype(np.int64)
    n_rescued = 0
    if mode == "fp22":
        mx = np.concatenate(
            [res.results[c]["mx"] for c in range(NCORES)], axis=0
        )
        margin = mx[:, 0] - mx[:, 1]
        sus = np.nonzero(margin < TAU)[0]
        n_rescued = len(sus)
        if n_rescued:
            bi, hw = np.divmod(sus, HW)
            hi, wi = np.divmod(hw, W)
            xs = np.ascontiguousarray(x[bi, :, hi, wi], dtype=np.float32)
            e2 = np.sum(emb.astype(np.float32) ** 2, axis=0, dtype=np.float32)
            d2 = (
                np.sum(xs * xs, axis=1, keepdims=True)
                - 2.0 * (xs @ emb.astype(np.float32))
                + e2[None, :]
            ).astype(np.float32)
            new_idx = np.argmin(d2, axis=1)
            changed = new_idx != idx[sus]
            if changed.any():
                rows = sus[changed]
                idx[rows] = new_idx[changed]
                q[rows] = emb.T[idx[rows]]
    out = q.reshape(B, H, W, D)
    out = np.ascontiguousarray(np.moveaxis(out, -1, 1))
    return out, idx, n_rescued


def kernel(x: np.ndarray, emb: np.ndarray) -> np.ndarray:
    in_maps = _prep_inputs(x, emb, MODE)
    res = _run_spmd(in_maps, MODE)
    out, _, _ = _assemble(res, x, emb, MODE)
    return out
